# revision 1
# baseline (speedup 1.0000x reference)
"""AdvancedGCN (3-layer GCNConv + global_mean_pool + linear) on 8 Trainium2
NeuronCores via Bass/Tile.

Design (v2):
  - 8-way dst-node sharding; per layer the projected, dinv[src]-prescaled
    feature table (bf16, [NTOT x 64]) lives in HBM, AllGathered across cores.
    Table rows are laid out HALF-major so the AllGather splits into two
    contiguous collectives; the first half overlaps the layer tail.
  - Self-loops are explicit edges (no separate "own contribution" path).
  - Edge messages fetched with dma_gather (256B descriptors, int16 quad
    indices, 4 residue classes from pair-view addressing).
  - Fixed K=8 slots per (dst, class) in RANK-major order: slot (j*128+dl)
    of chunk j holds dst dl's rank-j edge.  The sum over ranks is then a
    pure elementwise reduction: ONE DVE tensor_reduce per (window, class)
    replaces ~32 PE one-hot matmuls per block.
  - Overflow edges (rank>=8) go through tightly packed chunks with
    host-built 0/1 one-hot lhsT matmuls accumulating into PSUM.
  - Epilogue per block (DVE/Act): agg = classsum + psum_ovf; h =
    relu(dinv[dst]*agg + bias); then inline projection: PE transpose of h,
    matmul with W_next, dinv scale, batched write to the bounce buffer.
  - Graph mean-pool via one-hot matmul accumulation; AllReduce; linear.
"""
import os
import sys
import types

sys.path.insert(0, "/opt/trn_rl_repo")

import numpy as np
import ml_dtypes

BF = ml_dtypes.bfloat16


def _install_ntff_hook():
    """The agent image's antenv lacks axon_hooks; fabricate it so
    run_bass_kernel_spmd(trace=True) can capture NTFF profiles."""
    try:
        import antenv
    except ImportError:
        return
    if "antenv.axon_hooks" in sys.modules:
        return
    mod = types.ModuleType("antenv.axon_hooks")
    mod._hook = None

    def set_axon_ntff_profile_hook(h):
        mod._hook = h

    def get_axon_ntff_profile_hook():
        return mod._hook

    mod.set_axon_ntff_profile_hook = set_axon_ntff_profile_hook
    mod.get_axon_ntff_profile_hook = get_axon_ntff_profile_hook
    sys.modules["antenv.axon_hooks"] = mod
    antenv.axon_hooks = mod
    try:
        from trn_agent_boot.trn_boot import _ntff_profile_via_ctypes

        hook = _ntff_profile_via_ctypes("/opt/axon/libaxon_pjrt.so")
        if hook is not None:
            mod._hook = hook
    except Exception:
        pass


_install_ntff_hook()

import concourse.bass as bass
import concourse.bacc as bacc
import concourse.mybir as mybir
import concourse.tile as tile
from concourse._compat import cdiv
from concourse.library_config import mlp
from concourse.masks import make_identity

F32 = mybir.dt.float32
BF16 = mybir.dt.bfloat16
I16 = mybir.dt.int16
AF = mybir.ActivationFunctionType
OP = mybir.AluOpType
AX = mybir.AxisListType

CFG_FULL = dict(
    n_nodes=100000,
    n_graphs=256,
    d_in=128,
    hid=64,
    n_cls=10,
    n_cores=8,
    sw=4,
)

R = 4   # residue classes (pair-view + hoff addressing, int16 quad indices)
K = 8   # fixed rank slots per (dst, class)
SP_SPLIT = os.environ.get("GCN_SP", "0") == "1"  # single_packet experiment


# --------------------------------------------------------------------------
# Host preprocessing: integer index bookkeeping
# --------------------------------------------------------------------------

class T:
    """Template: program-shape constants + per-core input arrays."""


def preprocess(x, edge_index, batch, cfg):
    t = T()
    NC = cfg["n_cores"]
    N = cfg["n_nodes"]
    HID = cfg["hid"]
    NG = cfg["n_graphs"]
    SW = cfg["sw"]
    assert N % NC == 0
    NPC = N // NC
    RB = cdiv(NPC, 128)            # 98 real dst blocks per core
    BLOCKS = RB
    SH = (RB + 1) * 128            # + zero pad block
    HB = 48                        # blocks in table half A (windows 0..11)
    H1, H2 = HB * 128, SH - HB * 128
    NTOT = NC * SH
    assert NTOT % R == 0 and NTOT // R <= 32768

    t.cfg = cfg
    t.NPC, t.BLOCKS, t.SH, t.NTOT = NPC, BLOCKS, SH, NTOT
    t.HB, t.H1, t.H2 = HB, H1, H2
    t.GB = cdiv(NG, 128)
    t.sws = [list(range(i, min(i + SW, BLOCKS))) for i in range(0, BLOCKS, SW)]
    # p1 windows cover the pad block too (its xT cols are zero)
    t.p1w = [list(range(i, min(i + SW, BLOCKS + 1))) for i in range(0, BLOCKS + 1, SW)]

    src0 = edge_index[0].astype(np.int64)
    dst0 = edge_index[1].astype(np.int64)
    loops = np.arange(N, dtype=np.int64)
    src = np.concatenate([src0, loops])
    dst = np.concatenate([dst0, loops])
    deg = np.bincount(dst, minlength=N).astype(np.float32)  # includes loop

    c_s = src // NPC
    i_s = src % NPC
    grow = np.where(i_s < H1, c_s * H1 + i_s, NC * H1 + c_s * H2 + (i_s - H1))
    qidx = (grow // R).astype(np.int16)
    res = grow % R

    core = dst // NPC
    dloc = dst % NPC
    blk = dloc // 128
    dl = dloc % 128

    # rank of each edge within its (dst, class) group
    gk = dst * R + res
    order0 = np.argsort(gk, kind="stable")
    gk_s = gk[order0]
    grp_start = np.r_[0, np.nonzero(np.diff(gk_s))[0] + 1]
    sizes = np.diff(np.r_[grp_start, gk_s.size])
    rank_s = np.arange(gk_s.size) - np.repeat(grp_start, sizes)
    rank = np.empty_like(rank_s)
    rank[order0] = rank_s
    fixed_m = rank < K

    # overflow: per (core, blk, res); chunk counts shared across cores (max)
    ncells = NC * BLOCKS * R
    okey = ((core * BLOCKS + blk) * R + res)[~fixed_m]
    o_order = np.argsort(okey, kind="stable")
    oq = qidx[~fixed_m][o_order]
    odl = dl[~fixed_m][o_order]
    ocounts = np.bincount(okey, minlength=ncells).reshape(NC, BLOCKS, R)
    ostarts = np.zeros(ncells + 1, dtype=np.int64)
    np.cumsum(ocounts.reshape(-1), out=ostarts[1:])
    ovf_chunks = np.ceil(ocounts.max(axis=0) / 128).astype(np.int64)  # [BLOCKS, R]
    t.ovf_chunks = ovf_chunks

    # per (si, r): column layout = [fixed: b-major, K each][ovf: b-major]
    t.colsr = [[len(sw) * K + int(ovf_chunks[sw, r].sum()) for r in range(R)]
               for sw in t.sws]
    t.Lsr = [[c * 128 for c in cr] for cr in t.colsr]
    icol = []
    off = 0
    for si in range(len(t.sws)):
        icol.append([])
        for r in range(R):
            icol[si].append(off)
            off += t.Lsr[si][r] // 16
    t.icol, t.TOTC = icol, off

    # ovf chunk col within class region: nfix + cumsum over blocks before b
    t.ocol = {}
    for si, sw in enumerate(t.sws):
        for r in range(R):
            base = len(sw) * K
            for b in sw:
                t.ocol[(si, b, r)] = base
                base += int(ovf_chunks[b, r])

    # host one-hot layout: window-major; within window (b, r, j)
    t.dcol = {}
    off = 0
    for si, sw in enumerate(t.sws):
        t.dcol[si] = off
        for b in sw:
            for r in range(R):
                for j in range(int(ovf_chunks[b, r])):
                    t.dcol[(si, b, r, j)] = off
                    off += 1
    t.TOTI = off
    t.dwin = [sum(int(ovf_chunks[b, r]) for b in sw for r in range(R))
              for sw in t.sws]

    xT = x.T.astype(np.float32)  # [d_in, N]
    t.per_core = []
    for c in range(NC):
        padrow = NC * H1 + c * H2 + (RB * 128 - H1)  # first pad-block row
        padq = padrow // R
        m_c = core == c

        fidx = np.full((BLOCKS, R, K * 128), padq, dtype=np.int16)
        fm = m_c & fixed_m
        fidx[blk[fm], res[fm], rank[fm] * 128 + dl[fm]] = qidx[fm]

        gidx = np.zeros((128, t.TOTC), dtype=np.int16)
        ohm = np.zeros((max(t.TOTI, 1), 128, 128), dtype=BF)
        for si, sw in enumerate(t.sws):
            for r in range(R):
                parts = [fidx[b, r] for b in sw]
                for b in sw:
                    m = (c * BLOCKS + b) * R + r
                    s_, e_ = ostarts[m], ostarts[m + 1]
                    L = int(ovf_chunks[b, r]) * 128
                    qq = np.full(L, padq, dtype=np.int16)
                    qq[: e_ - s_] = oq[s_:e_]
                    parts.append(qq)
                    dd = odl[s_:e_]
                    for j in range(int(ovf_chunks[b, r])):
                        seg = dd[j * 128 : (j + 1) * 128]
                        ohm[t.dcol[(si, b, r, j)]][np.arange(seg.size), seg] = 1
                v = np.concatenate(parts)
                w = v.reshape(-1, 16).T
                gidx[:, icol[si][r] : icol[si][r] + v.size // 16] = np.tile(w, (8, 1))
        ohm = np.ascontiguousarray(ohm.swapaxes(0, 1).reshape(128, -1))

        lo, hi = c * NPC, (c + 1) * NPC
        xTs = np.zeros((cfg["d_in"], SH), dtype=np.float32)
        xTs[:, :NPC] = xT[:, lo:hi]

        degs = np.ones(SH, dtype=np.float32)
        degs[:NPC] = deg[lo:hi]
        degw = degs.reshape(-1, 128).T.copy()  # [128, RB+1]

        bats = np.full(SH, -1, dtype=np.int64)
        bats[:NPC] = batch[lo:hi]
        batw = bats.reshape(-1, 128).T  # [128, RB+1]
        ohg = np.zeros((128, BLOCKS * t.GB * 128), dtype=BF)
        for b in range(BLOCKS):
            for gb in range(t.GB):
                eq = batw[:, b : b + 1] == (gb * 128 + np.arange(128))[None, :]
                ohg[:, (b * t.GB + gb) * 128 : (b * t.GB + gb + 1) * 128] = eq
        t.per_core.append(dict(gidx=gidx, oh=ohm, ohg=ohg, xT=xTs, deg=degw))

    cnt = np.bincount(batch.astype(np.int64), minlength=NG).astype(np.float32)
    cntw = np.zeros((128, t.GB), dtype=np.float32)
    for gb in range(t.GB):
        n = min(128, NG - gb * 128)
        cntw[:n, gb] = cnt[gb * 128 : gb * 128 + n]
    t.cnt = cntw
    return t


def make_in_maps(t, W1, b1, W2, b2, W3, b3, Wlin, blin):
    shared = dict(
        cnt=t.cnt,
        w1=W1.astype(np.float32),
        w2b=W2.astype(BF),
        w3b=W3.astype(BF),
        wlb=Wlin.astype(BF),
        b1t=np.tile(b1.astype(np.float32), (128, 1)),
        b2t=np.tile(b2.astype(np.float32), (128, 1)),
        b3t=np.tile(b3.astype(np.float32), (128, 1)),
        blt=np.tile(blin.astype(np.float32), (128, 1)),
    )
    return [dict(shared, **pc) for pc in t.per_core]


# --------------------------------------------------------------------------
# Device program
# --------------------------------------------------------------------------

def build_program(t):
    cfg = t.cfg
    NC = cfg["n_cores"]
    HID = cfg["hid"]
    DIN = cfg["d_in"]
    NG = cfg["n_graphs"]
    NCLS = cfg["n_cls"]
    BLOCKS, SH, NTOT, GB = t.BLOCKS, t.SH, t.NTOT, t.GB
    HB, H1, H2 = t.HB, t.H1, t.H2
    colsr, Lsr, icol = t.colsr, t.Lsr, t.icol
    ovf_chunks = t.ovf_chunks
    NW = len(t.sws)
    col_max = max(max(cr) for cr in colsr)
    dw_max = max(t.dwin)
    ix_max = max(max(Lsr[si][r] // 16 for r in range(R)) for si in range(len(t.sws)))

    nc = bacc.Bacc(
        "TRN2",
        target_bir_lowering=False,
        debug=False,
        enable_asserts=False,
        num_devices=NC,
        num_swdge_queues=4,
    )

    din = lambda n, s, d=F32: nc.dram_tensor(n, s, d, kind="ExternalInput")
    xT_d = din("xT", [DIN, SH])
    gidx_d = din("gidx", [128, t.TOTC], I16)
    ohd = din("oh", [128, max(t.TOTI, 1) * 128], BF16)
    deg_d = din("deg", [128, BLOCKS + 1])
    ohg_d = din("ohg", [128, BLOCKS * GB * 128], BF16)
    cnt_d = din("cnt", [128, GB])
    w1_d = din("w1", [DIN, HID])
    w2b_d = din("w2b", [HID, HID], BF16)
    w3b_d = din("w3b", [HID, HID], BF16)
    wlb_d = din("wlb", [HID, NCLS], BF16)
    b1t_d = din("b1t", [128, HID])
    b2t_d = din("b2t", [128, HID])
    b3t_d = din("b3t", [128, HID])
    blt_d = din("blt", [128, NCLS])
    out_d = nc.dram_tensor("out", [NG, NCLS], F32, kind="ExternalOutput")

    tab = [nc.dram_tensor(f"table{k}", [NTOT, HID], BF16, addr_space="Shared") for k in range(3)]
    bncA = [nc.dram_tensor(f"bncA{k}", [H1, HID], BF16) for k in range(3)]
    bncB = [nc.dram_tensor(f"bncB{k}", [H2, HID], BF16) for k in range(3)]
    pool_loc = nc.dram_tensor("pool_loc", [128 * GB, HID], F32)
    pool_sum = nc.dram_tensor("pool_sum", [128 * GB, HID], F32, addr_space="Shared")

    groups = [list(range(NC))]

    def ag_half(k, half):
        if half == 0:
            ins, outs = bncA[k], tab[k].ap()[0 : NC * H1, :]
        else:
            ins, outs = bncB[k], tab[k].ap()[NC * H1 : NTOT, :]
        nc.gpsimd.collective_compute(
            "AllGather", OP.bypass, replica_groups=groups,
            ins=[ins.ap().opt()], outs=[outs.opt()],
        )

    def bnc_write(k, b0, nb, tbb):
        # tbb: [128, nb, HID] bf16, block jb rows -> bnc rows (b0+jb)*128..
        row0 = b0 * 128
        if b0 < HB:
            dst = bncA[k].ap()[row0 : row0 + nb * 128, :]
        else:
            dst = bncB[k].ap()[row0 - H1 : row0 - H1 + nb * 128, :]
        nc.sync.dma_start(dst.rearrange("(b p) h -> p b h", b=nb), tbb[:, :nb, :])

    with tile.TileContext(nc) as tc:
        with (
            tc.tile_pool(name="const", bufs=1) as cp,
            tc.tile_pool(name="xw", bufs=2) as xp,
            tc.tile_pool(name="ix", bufs=4) as ixp,
            tc.tile_pool(name="m0", bufs=2) as mp0,
            tc.tile_pool(name="m1", bufs=2) as mp1,
            tc.tile_pool(name="m2", bufs=2) as mp2,
            tc.tile_pool(name="m3", bufs=2) as mp3,
            tc.tile_pool(name="oh", bufs=4) as ohp,
            tc.tile_pool(name="win", bufs=3) as wp,
            tc.tile_pool(name="ep", bufs=3) as ep,
            tc.tile_pool(name="tb", bufs=3) as tbp,
            tc.tile_pool(name="psb", bufs=2, space="PSUM") as psb,
            tc.tile_pool(name="pst", bufs=2, space="PSUM") as pst,
            tc.tile_pool(name="psw", bufs=2, space="PSUM") as psw,
            tc.tile_pool(name="psg", bufs=1, space="PSUM") as psg,
        ):
            nc.gpsimd.load_library(mlp)
            mps = [mp0, mp1, mp2, mp3]

            # ---- constants ----
            cnt_t = cp.tile([128, GB], F32, tag="cnt", name="cnt")
            nc.sync.dma_start(cnt_t[:], cnt_d[:, :])
            deg_t = cp.tile([128, BLOCKS + 1], F32, tag="deg", name="deg")
            nc.sync.dma_start(deg_t[:], deg_d[:, :])
            dsq_t = cp.tile([128, BLOCKS + 1], F32, tag="dsq", name="dsq")
            nc.scalar.activation(dsq_t[:], deg_t[:], AF.Sqrt)
            dinv_t = cp.tile([128, BLOCKS + 1], F32, tag="dinv", name="dinv")
            nc.vector.reciprocal(dinv_t[:], dsq_t[:])
            w1_t = cp.tile([DIN, HID], F32, tag="w1", name="w1")
            nc.sync.dma_start(w1_t[:], w1_d[:, :])
            w2b_t = cp.tile([HID, HID], BF16, tag="w2b", name="w2b")
            nc.sync.dma_start(w2b_t[:], w2b_d[:, :])
            w3b_t = cp.tile([HID, HID], BF16, tag="w3b", name="w3b")
            nc.sync.dma_start(w3b_t[:], w3b_d[:, :])
            wlb_t = cp.tile([HID, NCLS], BF16, tag="wlb", name="wlb")
            nc.sync.dma_start(wlb_t[:], wlb_d[:, :])
            bt = []
            for nm, d in (("b1t", b1t_d), ("b2t", b2t_d), ("b3t", b3t_d)):
                b_ = cp.tile([128, HID], F32, tag=nm, name=nm)
                nc.sync.dma_start(b_[:], d[:, :])
                bt.append(b_)
            blt_t = cp.tile([128, NCLS], F32, tag="blt", name="blt")
            nc.sync.dma_start(blt_t[:], blt_d[:, :])
            identb = cp.tile([128, 128], BF16, tag="identb", name="identb")
            make_identity(nc, identb[:])
            hall = cp.tile([128, BLOCKS, HID], BF16, tag="hall", name="hall")
            wnextb = [w2b_t, w3b_t]

            # ---- phase 1: table0 = (x @ W1) * dinv ----
            with nc.named_scope("p1"):
                for si, sw in enumerate(t.p1w):
                    nb = len(sw)
                    b0 = sw[0]
                    xt = xp.tile([DIN, 4 * 128], F32, tag="xt", name="xt")
                    nc.sync.dma_start(xt[:, : nb * 128], xT_d[:, b0 * 128 : (b0 + nb) * 128])
                    tbb = tbp.tile([128, 4, HID], BF16, tag="tbb", name="tbb")
                    for jb, b in enumerate(sw):
                        ps = psw.tile([128, HID], F32, tag="psw", name="psw")
                        nc.tensor.matmul(ps[:], lhsT=xt[:, jb * 128 : (jb + 1) * 128],
                                         rhs=w1_t[:], start=True, stop=True)
                        nc.scalar.activation(tbb[:, jb, :], ps[:], AF.Copy,
                                             scale=dinv_t[:, b : b + 1])
                    bnc_write(0, b0, nb, tbb)
                    if b0 + nb == HB:
                        ag_half(0, 0)
                ag_half(0, 1)

            # ---- layers ----
            pool_ps = None
            state = {}

            def dispatch(k, si, rviews):
                sw = t.sws[si]
                ixr = ixp.tile([128, max(4 * ix_max, 16)], I16, tag="ix", name="ix")
                c0 = icol[si][0]
                cend = icol[si][R - 1] + Lsr[si][R - 1] // 16
                nc.sync.dma_start(ixr[:, : cend - c0], gidx_d[:, c0:cend])
                dw = t.dwin[si]
                ohl = None
                if dw:
                    ohl = ohp.tile([128, max(dw_max, 1) * 128], BF16, tag="ohl", name="ohl")
                    d0 = t.dcol[si]
                    nc.sync.dma_start(ohl[:, : dw * 128], ohd[:, d0 * 128 : (d0 + dw) * 128])
                gts = [None] * R
                for r in (1, 2, 3, 0):
                    L = Lsr[si][r]
                    cols = colsr[si][r]
                    gt = mps[r].tile([128, col_max, 2 * HID], BF16, tag=f"m{r}", name=f"m{r}")
                    io = icol[si][r] - c0
                    if SP_SPLIT:
                        for t0 in range(0, cols, 8):
                            cn = min(8, cols - t0)
                            nc.gpsimd.dma_gather(
                                gt[:, t0 : t0 + cn, :], rviews[r],
                                ixr[:, io + t0 * 8 : io + t0 * 8 + cn * 8],
                                cn * 128, cn * 128, 2 * HID, elem_step=R * HID,
                                single_packet=True, queue_num=r,
                            )
                    else:
                        nc.gpsimd.dma_gather(
                            gt[:, :cols, :], rviews[r], ixr[:, io : io + L // 16],
                            L, L, 2 * HID, elem_step=R * HID,
                            single_packet=False, queue_num=r,
                        )
                    gts[r] = gt
                state[si] = (gts, ohl)

            def process(k, si):
                sw = t.sws[si]
                nb = len(sw)
                gts, ohl = state.pop(si)
                nfix = nb * K
                # class reductions: [128, nb, 64] f32 per class
                cs = []
                for r in range(R):
                    hoff = (r % 2) * HID
                    csr = wp.tile([128, 4 * HID], F32, tag=f"cs{r}", name=f"cs{r}")
                    view = gts[r][:, 0:nfix, hoff : hoff + HID].rearrange(
                        "p (b j) h -> p b h j", j=K)
                    nc.vector.tensor_reduce(
                        csr[:, : nb * HID].rearrange("p (b h) -> p b h", b=nb),
                        view, axis=AX.X, op=OP.add)
                    cs.append(csr)
                s01 = wp.tile([128, 4 * HID], F32, tag="s01", name="s01")
                nc.vector.tensor_tensor(out=s01[:, : nb * HID], in0=cs[0][:, : nb * HID],
                                        in1=cs[1][:, : nb * HID], op=OP.add)
                s23 = wp.tile([128, 4 * HID], F32, tag="s23", name="s23")
                nc.vector.tensor_tensor(out=s23[:, : nb * HID], in0=cs[2][:, : nb * HID],
                                        in1=cs[3][:, : nb * HID], op=OP.add)
                s = wp.tile([128, 4 * HID], F32, tag="s", name="s")
                nc.vector.tensor_tensor(out=s[:, : nb * HID], in0=s01[:, : nb * HID],
                                        in1=s23[:, : nb * HID], op=OP.add)

                for jb, b in enumerate(sw):
                    novf = int(ovf_chunks[b, :].sum())
                    ps = None
                    if novf:
                        ps = psb.tile([128, HID], F32, tag="psb", name="psb")
                        done = 0
                        for r in range(R):
                            ch = int(ovf_chunks[b, r])
                            if ch == 0:
                                continue
                            hoff = (r % 2) * HID
                            oc = t.ocol[(si, b, r)]
                            for j in range(ch):
                                lo = (t.dcol[(si, b, r, j)] - t.dcol[si]) * 128
                                nc.tensor.matmul(
                                    ps[:], lhsT=ohl[:, lo : lo + 128],
                                    rhs=gts[r][:, oc + j, hoff : hoff + HID],
                                    start=(done == 0), stop=(done == novf - 1))
                                done += 1
                    sl = s[:, jb * HID : (jb + 1) * HID]
                    if novf:
                        agg = ep.tile([128, HID], F32, tag="agg", name="agg")
                        nc.vector.tensor_tensor(out=agg[:], in0=sl, in1=ps[:], op=OP.add)
                        aggap = agg[:]
                    else:
                        aggap = sl
                    t1 = ep.tile([128, HID], F32, tag="t1", name="t1")
                    nc.scalar.activation(t1[:], aggap, AF.Copy, scale=dinv_t[:, b : b + 1])
                    hp = ep.tile([128, HID], F32, tag="hp", name="hp")
                    nc.vector.tensor_tensor(out=hp[:], in0=t1[:], in1=bt[k][:], op=OP.add)
                    nc.scalar.activation(hall[:, b, :], hp[:],
                                         AF.Relu if k < 2 else AF.Copy)

            def projwin(k, si):
                sw = t.sws[si]
                nb = len(sw)
                if k < 2:
                    tbb = tbp.tile([128, 4, HID], BF16, tag="tbb", name="tbb")
                    for jb, b in enumerate(sw):
                        pt = pst.tile([128, 128], BF16, tag="pstb", name="pstb")
                        nc.tensor.transpose(pt[:HID, :], hall[:, b, :], identb[:])
                        hT = ep.tile([HID, 128], BF16, tag="hT", name="hT")
                        nc.scalar.copy(hT[:], pt[:HID, :])
                        ps2 = psw.tile([128, HID], F32, tag="psw", name="psw")
                        nc.tensor.matmul(ps2[:], lhsT=hT[:], rhs=wnextb[k][:],
                                         start=True, stop=True)
                        nc.scalar.activation(tbb[:, jb, :], ps2[:], AF.Copy,
                                             scale=dinv_t[:, b : b + 1])
                    bnc_write(k + 1, sw[0], nb, tbb)
                else:
                    ohgl = ohp.tile([128, 4 * GB * 128], BF16, tag="ohgl", name="ohgl")
                    b0 = sw[0]
                    nc.sync.dma_start(ohgl[:, : nb * GB * 128],
                                      ohg_d[:, b0 * GB * 128 : (b0 + nb) * GB * 128])
                    for jb, b in enumerate(sw):
                        for gb in range(GB):
                            gp = min(128, NG - gb * 128)
                            nc.tensor.matmul(
                                pool_ps[gb][:gp, :],
                                lhsT=ohgl[:, (jb * GB + gb) * 128 : (jb * GB + gb) * 128 + gp],
                                rhs=hall[:, b, :],
                                start=(b == 0), stop=(b == BLOCKS - 1))

            for k in range(3):
                tview = tab[k].ap().rearrange("(a b) d -> a (b d)", b=R)
                rviews = [tview[:, (r // 2) * 2 * HID : (r // 2 + 1) * 2 * HID] for r in range(R)]
                if k == 2:
                    pool_ps = [psg.tile([128, HID], F32, tag=f"psg{gb}", name=f"psg{gb}") for gb in range(GB)]

                sid, _ = nc.enter_named_scope(f"L{k}", notify=False)
                D = 2
                for si in range(NW):
                    dispatch(k, si, rviews)
                    if si > 0:
                        process(k, si - 1)
                    if si - D >= 0:
                        projwin(k, si - D)
                    if k < 2 and si - D == 11:
                        ag_half(k + 1, 0)  # blocks 0..47 projected by windows 0..11
                process(k, NW - 1)
                for si in range(NW - D, NW):
                    projwin(k, si)
                nc.leave_named_scope(f"L{k}", sid, notify=False)
                if k < 2:
                    zt = ep.tile([128, HID], BF16, tag="h", name="h")
                    nc.any.memset(zt[:], 0.0)
                    nc.sync.dma_start(bncB[k + 1][H2 - 128 : H2, :], zt[:])
                    with nc.named_scope(f"ag{k+1}"):
                        ag_half(k + 1, 1)

            # ---- pooling epilogue + final linear ----
            tid, _ = nc.enter_named_scope("tail", notify=False)
            for gb in range(GB):
                gp = min(128, NG - gb * 128)
                cpt = ep.tile([128, HID], F32, tag="t1", name="t1")
                if gp < 128:
                    nc.any.memset(cpt[:], 0.0)
                nc.vector.tensor_copy(out=cpt[:gp, :], in_=pool_ps[gb][:gp, :])
                nc.sync.dma_start(pool_loc[gb * 128 : (gb + 1) * 128, :], cpt[:])
            nc.gpsimd.collective_compute(
                "AllReduce", OP.add, replica_groups=groups,
                ins=[pool_loc.ap().opt()], outs=[pool_sum.ap().opt()],
            )
            mx_t = ep.tile([128, GB], F32, tag="mx", name="mx")
            nc.vector.tensor_scalar(mx_t[:], cnt_t[:], 1.0, None, OP.max)
            inv_t = ep.tile([128, GB], F32, tag="inv", name="inv")
            nc.vector.reciprocal(inv_t[:], mx_t[:])
            for gb in range(GB):
                gp = min(128, NG - gb * 128)
                sm = ep.tile([128, HID], F32, tag="t1", name="t1")
                nc.sync.dma_start(sm[:], pool_sum[gb * 128 : (gb + 1) * 128, :])
                mean = ep.tile([128, HID], BF16, tag="h", name="h")
                nc.vector.tensor_scalar(mean[:], sm[:], inv_t[:, gb : gb + 1], None, OP.mult)
                pt = pst.tile([128, 128], BF16, tag="pstb", name="pstb")
                nc.tensor.transpose(pt[:HID, :], mean[:], identb[:])
                mT = ep.tile([HID, 128], BF16, tag="hT", name="hT")
                nc.scalar.copy(mT[:], pt[:HID, :])
                psf = psw.tile([128, NCLS], F32, tag="psw", name="psw")
                nc.tensor.matmul(psf[:gp, :], lhsT=mT[:, :gp], rhs=wlb_t[:], start=True, stop=True)
                of = ep.tile([128, NCLS], F32, tag="of", name="of")
                nc.vector.tensor_tensor(out=of[:gp, :], in0=psf[:gp, :], in1=blt_t[:gp, :], op=OP.add)
                nc.sync.dma_start(out_d[gb * 128 : gb * 128 + gp, :], of[:gp, :])
            nc.leave_named_scope("tail", tid, notify=False)

    if os.environ.get("GCN_BUILD_ONLY", "0") != "1":
        nc.compile()
    return nc


# --------------------------------------------------------------------------
# Entry points
# --------------------------------------------------------------------------

def run_on_hw(inputs, cfg, trace=None):
    from concourse.bass_utils import run_bass_kernel_spmd

    if trace is None:
        trace = os.environ.get("GCN_TRACE", "0") == "1"
    t = preprocess(np.asarray(inputs["x"]), np.asarray(inputs["edge_index"]),
                   np.asarray(inputs["batch"]), cfg)
    in_maps = make_in_maps(
        t, *(np.asarray(inputs[k]) for k in
             ("W1", "b1", "W2", "b2", "W3", "b3", "Wlin", "blin")))
    nc = build_program(t)
    res = run_bass_kernel_spmd(nc, in_maps, core_ids=list(range(cfg["n_cores"])), trace=trace)
    run_on_hw.last = res
    return res.results[0]["out"].astype(np.float32)


def kernel(**inputs) -> np.ndarray:
    return run_on_hw(inputs, CFG_FULL)



# revision 2
# speedup vs baseline: 1.0076x; 1.0076x over previous
"""AdvancedGCN on 8 Trainium2 NeuronCores via Bass/Tile — v3.

Key structural changes vs v2 baseline:
  - Layer 3 + global mean pool collapsed algebraically: since gcn3 has no
    relu and feeds straight into mean-pool, pooled_sum[g] =
    sum_s Wg[g,s] * table2[s] with Wg[g,s] = sum_{edges s->d, batch[d]=g}
    dinv[d] (+ self loop) — a host-precomputed structure-only matrix.
    Each core only needs its OWN shard of table2, so the third AllGather
    disappears and layer 3 becomes 196 accumulating matmuls.
  - Self-loop edges removed from the gather stream; own contribution added
    in the epilogue from the locally-kept projected rows (tabloc).
  - Overflow (rank>=K) aggregation via on-chip one-hot masks (DVE iota
    is_equal) + PE matmuls; no host one-hot tables, no HBM mask loads.
  - K=7 rank slots per (dst, residue-class).
"""
import os
import sys
import types

sys.path.insert(0, "/opt/trn_rl_repo")

import numpy as np
import ml_dtypes

BF = ml_dtypes.bfloat16


def _install_ntff_hook():
    try:
        import antenv
    except ImportError:
        return
    if "antenv.axon_hooks" in sys.modules:
        return
    mod = types.ModuleType("antenv.axon_hooks")
    mod._hook = None

    def set_axon_ntff_profile_hook(h):
        mod._hook = h

    def get_axon_ntff_profile_hook():
        return mod._hook

    mod.set_axon_ntff_profile_hook = set_axon_ntff_profile_hook
    mod.get_axon_ntff_profile_hook = get_axon_ntff_profile_hook
    sys.modules["antenv.axon_hooks"] = mod
    antenv.axon_hooks = mod
    try:
        from trn_agent_boot.trn_boot import _ntff_profile_via_ctypes

        hook = _ntff_profile_via_ctypes("/opt/axon/libaxon_pjrt.so")
        if hook is not None:
            mod._hook = hook
    except Exception:
        pass


_install_ntff_hook()

import concourse.bass as bass
import concourse.bacc as bacc
import concourse.mybir as mybir
import concourse.tile as tile
from concourse._compat import cdiv
from concourse.library_config import mlp
from concourse.masks import make_identity

F32 = mybir.dt.float32
BF16 = mybir.dt.bfloat16
I16 = mybir.dt.int16
AF = mybir.ActivationFunctionType
OP = mybir.AluOpType
AX = mybir.AxisListType

CFG_FULL = dict(
    n_nodes=100000,
    n_graphs=256,
    d_in=128,
    hid=64,
    n_cls=10,
    n_cores=8,
    sw=4,
)

R = 4   # residue classes (pair-view + hoff addressing, int16 quad indices)
K = 7   # fixed rank slots per (dst, class)
PAD_DL = 200.0  # one-hot-killing dst-local for ovf padding slots


# --------------------------------------------------------------------------
# Host preprocessing
# --------------------------------------------------------------------------

class T:
    pass


def preprocess(x, edge_index, batch, cfg):
    t = T()
    NC = cfg["n_cores"]
    N = cfg["n_nodes"]
    NG = cfg["n_graphs"]
    SW = cfg["sw"]
    assert N % NC == 0
    NPC = N // NC
    RB = cdiv(NPC, 128)            # 98 real dst blocks per core
    BLOCKS = RB
    SH = (RB + 1) * 128            # + zero pad block
    HB = 48
    HB2 = 80
    H1, H2 = HB * 128, (HB2 - HB) * 128
    NTOT = NC * SH
    assert NTOT % R == 0 and NTOT // R <= 32768

    t.cfg = cfg
    t.NPC, t.BLOCKS, t.SH, t.NTOT = NPC, BLOCKS, SH, NTOT
    t.HB, t.H1, t.H2 = HB, H1, H2
    # three AllGather segments (block-aligned, window-aligned)
    H3 = SH - H1 - H2
    t.H3 = H3
    t.GB = cdiv(NG, 128)
    t.sws = [list(range(i, min(i + SW, BLOCKS))) for i in range(0, BLOCKS, SW)]

    src = edge_index[0].astype(np.int64)
    dst = edge_index[1].astype(np.int64)
    deg = (np.bincount(dst, minlength=N) + 1.0).astype(np.float64)  # + self loop
    dinv = 1.0 / np.sqrt(deg)

    def growmap(c, i):
        return np.where(
            i < H1, c * H1 + i,
            np.where(i < H1 + H2, NC * H1 + c * H2 + (i - H1),
                     NC * (H1 + H2) + c * H3 + (i - H1 - H2)))

    c_s = src // NPC
    i_s = src % NPC
    grow = growmap(c_s, i_s)
    qidx = (grow // R).astype(np.int16)
    res = grow % R

    core = dst // NPC
    dloc = dst % NPC
    blk = dloc // 128
    dl = dloc % 128

    # rank of each edge within its (dst, class) group
    gk = dst * R + res
    order0 = np.argsort(gk, kind="stable")
    gk_s = gk[order0]
    grp_start = np.r_[0, np.nonzero(np.diff(gk_s))[0] + 1]
    sizes = np.diff(np.r_[grp_start, gk_s.size])
    rank_s = np.arange(gk_s.size) - np.repeat(grp_start, sizes)
    rank = np.empty_like(rank_s)
    rank[order0] = rank_s
    fixed_m = rank < K

    # overflow per (core, blk, res); chunk counts shared across cores (max)
    ncells = NC * BLOCKS * R
    okey = ((core * BLOCKS + blk) * R + res)[~fixed_m]
    o_order = np.argsort(okey, kind="stable")
    oq = qidx[~fixed_m][o_order]
    odl = dl[~fixed_m][o_order]
    ocounts = np.bincount(okey, minlength=ncells).reshape(NC, BLOCKS, R)
    ostarts = np.zeros(ncells + 1, dtype=np.int64)
    np.cumsum(ocounts.reshape(-1), out=ostarts[1:])
    ovf_chunks = np.ceil(ocounts.max(axis=0) / 128).astype(np.int64)  # [BLOCKS, R]
    t.ovf_chunks = ovf_chunks

    # per (si, r): columns = [fixed: b-major, K each][ovf: b-major chunks]
    t.colsr = [[len(sw) * K + int(ovf_chunks[sw, r].sum()) for r in range(R)]
               for sw in t.sws]
    t.Lsr = [[c * 128 for c in cr] for cr in t.colsr]
    icol = []
    off = 0
    for si in range(len(t.sws)):
        icol.append([])
        for r in range(R):
            icol[si].append(off)
            off += t.Lsr[si][r] // 16
    t.icol, t.TOTC = icol, off

    # ovf chunk col within class tile: nfix + cumsum over blocks before b
    t.ocol = {}
    for si, sw in enumerate(t.sws):
        for r in range(R):
            base = len(sw) * K
            for b in sw:
                t.ocol[(si, b, r)] = base
                base += int(ovf_chunks[b, r])

    # dlo (ovf dst-local values, f32) layout: window-major; within: (r, b, j)
    t.dcol = {}
    t.rcol = {}
    off = 0
    for si, sw in enumerate(t.sws):
        t.dcol[si] = off
        for r in range(R):
            t.rcol[(si, r)] = off
            for b in sw:
                for j in range(int(ovf_chunks[b, r])):
                    t.dcol[(si, b, r, j)] = off
                    off += 1
            t.rcol[(si, r, "n")] = off - t.rcol[(si, r)]
    t.DLOTOT = max(off, 1)
    t.dwin = [sum(int(ovf_chunks[b, r]) for b in sw for r in range(R))
              for sw in t.sws]

    # Wg: pooled layer-3 weights. Wg[grow_s, g] = sum_{edges s->d, batch[d]=g}
    # dinv[d]  (+ self loop term dinv[s] at g=batch[s])
    bat = batch.astype(np.int64)
    wg = np.zeros((NTOT, NG), dtype=np.float64)
    np.add.at(wg, (grow, bat[dst]), dinv[dst])
    ii = np.arange(N)
    grow_all = growmap(ii // NPC, ii % NPC)
    np.add.at(wg, (grow_all, bat), dinv)

    xT = x.T.astype(np.float32)  # [d_in, N]
    t.per_core = []
    for c in range(NC):
        padrow = NC * (H1 + H2) + c * H3 + (RB * 128 - H1 - H2)  # first pad row
        padq = padrow // R
        m_c = core == c

        fidx = np.full((BLOCKS, R, K * 128), padq, dtype=np.int16)
        fm = m_c & fixed_m
        fidx[blk[fm], res[fm], rank[fm] * 128 + dl[fm]] = qidx[fm]

        gidx = np.zeros((128, t.TOTC), dtype=np.int16)
        dlo = np.full((128, t.DLOTOT), PAD_DL, dtype=np.float32)
        for si, sw in enumerate(t.sws):
            for r in range(R):
                parts = [fidx[b, r] for b in sw]
                for b in sw:
                    m = (c * BLOCKS + b) * R + r
                    s_, e_ = ostarts[m], ostarts[m + 1]
                    Lb = int(ovf_chunks[b, r]) * 128
                    qq = np.full(Lb, padq, dtype=np.int16)
                    qq[: e_ - s_] = oq[s_:e_]
                    parts.append(qq)
                    dd = odl[s_:e_]
                    for j in range(int(ovf_chunks[b, r])):
                        seg = dd[j * 128: (j + 1) * 128]
                        col = t.dcol[(si, b, r, j)]
                        dlo[: seg.size, col] = seg.astype(np.float32)
                v = np.concatenate(parts)
                w = v.reshape(-1, 16).T
                gidx[:, icol[si][r]: icol[si][r] + v.size // 16] = np.tile(w, (8, 1))

        lo, hi = c * NPC, (c + 1) * NPC
        xTs = np.zeros((cfg["d_in"], SH), dtype=np.float32)
        xTs[:, :NPC] = xT[:, lo:hi]

        degs = np.ones(SH, dtype=np.float32)
        degs[:NPC] = deg[lo:hi]
        degw = degs.reshape(-1, 128).T.copy()  # [128, RB+1]

        # WgT for this core's shard, ordered by shard-local row index
        wgc = np.concatenate([
            wg[c * H1:(c + 1) * H1],
            wg[NC * H1 + c * H2: NC * H1 + (c + 1) * H2],
            wg[NC * (H1 + H2) + c * H3: NC * (H1 + H2) + (c + 1) * H3]])
        t.per_core.append(dict(gidx=gidx, dlo=dlo, xT=xTs, deg=degw,
                               wgt=wgc.astype(np.float32)))

    cnt = np.bincount(bat, minlength=NG).astype(np.float32)
    invc = (1.0 / np.maximum(cnt, 1.0)).astype(np.float32)
    invw = np.zeros((128, t.GB), dtype=np.float32)
    for gb in range(t.GB):
        n = min(128, NG - gb * 128)
        invw[:n, gb] = invc[gb * 128: gb * 128 + n]
    t.invc = invw

    iota = np.tile(np.arange(128, dtype=np.float32)[None, :], (128, 1))
    t.iota = iota
    return t


def make_in_maps(t, W1, b1, W2, b2, W3, b3, Wlin, blin):
    shared = dict(
        invc=t.invc,
        iota=t.iota,
        w1=W1.astype(np.float32),
        w2b=W2.astype(BF),
        w3b=W3.astype(BF),
        wlb=Wlin.astype(BF),
        b1t=np.tile(b1.astype(np.float32), (128, 4)),
        b2t=np.tile(b2.astype(np.float32), (128, 4)),
        b3t=np.tile(b3.astype(np.float32), (128, 1)),
        blt=np.tile(blin.astype(np.float32), (128, 1)),
    )
    return [dict(shared, **pc) for pc in t.per_core]


# --------------------------------------------------------------------------
# Device program
# --------------------------------------------------------------------------

def build_program(t):
    cfg = t.cfg
    NC = cfg["n_cores"]
    HID = cfg["hid"]
    DIN = cfg["d_in"]
    NG = cfg["n_graphs"]
    NCLS = cfg["n_cls"]
    BLOCKS, SH, NTOT, GB = t.BLOCKS, t.SH, t.NTOT, t.GB
    HB, H1, H2 = t.HB, t.H1, t.H2
    colsr, Lsr, icol = t.colsr, t.Lsr, t.icol
    ovf_chunks = t.ovf_chunks
    NW = len(t.sws)
    col_max = max(max(cr) for cr in colsr)
    dw_max = max(max(t.dwin), 1)
    ix_max = max(max(Lsr[si][r] // 16 for r in range(R)) for si in range(len(t.sws)))

    nc = bacc.Bacc(
        "TRN2",
        target_bir_lowering=False,
        debug=False,
        enable_asserts=False,
        num_devices=NC,
        num_swdge_queues=4,
    )

    din = lambda n, s, d=F32: nc.dram_tensor(n, s, d, kind="ExternalInput")
    xT_d = din("xT", [DIN, SH])
    gidx_d = din("gidx", [128, t.TOTC], I16)
    dlo_d = din("dlo", [128, t.DLOTOT])
    deg_d = din("deg", [128, BLOCKS + 1])
    wgt_d = din("wgt", [SH, NG], BF16)
    invc_d = din("invc", [128, GB])
    iota_d = din("iota", [128, 128])
    w1_d = din("w1", [DIN, HID])
    w2b_d = din("w2b", [HID, HID], BF16)
    w3b_d = din("w3b", [HID, HID], BF16)
    wlb_d = din("wlb", [HID, NCLS], BF16)
    b1t_d = din("b1t", [128, HID])
    b2t_d = din("b2t", [128, HID])
    b3t_d = din("b3t", [128, HID])
    blt_d = din("blt", [128, NCLS])
    out_d = nc.dram_tensor("out", [NG, NCLS], F32, kind="ExternalOutput")

    tab = [nc.dram_tensor(f"table{k}", [NTOT, HID], BF16, addr_space="Shared") for k in range(2)]
    bncA = [nc.dram_tensor(f"bncA{k}", [H1, HID], BF16) for k in range(2)]
    bncB = [nc.dram_tensor(f"bncB{k}", [H2, HID], BF16) for k in range(2)]
    pool_loc = nc.dram_tensor("pool_loc", [128 * GB, HID], F32)
    pool_sum = nc.dram_tensor("pool_sum", [128 * GB, HID], F32, addr_space="Shared")

    groups = [list(range(NC))]

    def ag_half(k, half):
        if half == 0:
            ins, outs = bncA[k], tab[k].ap()[0: NC * H1, :]
        else:
            ins, outs = bncB[k], tab[k].ap()[NC * H1: NTOT, :]
        nc.gpsimd.collective_compute(
            "AllGather", OP.bypass, replica_groups=groups,
            ins=[ins.ap().opt()], outs=[outs.opt()],
        )

    with tile.TileContext(nc) as tc:
        with (
            tc.tile_pool(name="const", bufs=1) as cp,
            tc.tile_pool(name="xw", bufs=2) as xp,
            tc.tile_pool(name="ix", bufs=4) as ixp,
            tc.tile_pool(name="m0", bufs=2) as mp0,
            tc.tile_pool(name="m1", bufs=2) as mp1,
            tc.tile_pool(name="m2", bufs=2) as mp2,
            tc.tile_pool(name="m3", bufs=2) as mp3,
            tc.tile_pool(name="oh", bufs=4) as ohp,
            tc.tile_pool(name="wg", bufs=3) as wgp,
            tc.tile_pool(name="win", bufs=3) as wp,
            tc.tile_pool(name="ep", bufs=3) as ep,
            tc.tile_pool(name="psb", bufs=2, space="PSUM") as psb,
            tc.tile_pool(name="pst", bufs=2, space="PSUM") as pst,
            tc.tile_pool(name="psw", bufs=2, space="PSUM") as psw,
            tc.tile_pool(name="psg", bufs=1, space="PSUM") as psg,
        ):
            nc.gpsimd.load_library(mlp)
            mps = [mp0, mp1, mp2, mp3]

            # ---- constants ----
            invc_t = cp.tile([128, GB], F32, tag="invc", name="invc")
            nc.sync.dma_start(invc_t[:], invc_d[:, :])
            iota_t = cp.tile([128, 128], F32, tag="iota", name="iota")
            nc.sync.dma_start(iota_t[:], iota_d[:, :])
            deg_t = cp.tile([128, BLOCKS + 1], F32, tag="deg", name="deg")
            nc.sync.dma_start(deg_t[:], deg_d[:, :])
            dsq_t = cp.tile([128, BLOCKS + 1], F32, tag="dsq", name="dsq")
            nc.scalar.activation(dsq_t[:], deg_t[:], AF.Sqrt)
            dinv_t = cp.tile([128, BLOCKS + 1], F32, tag="dinv", name="dinv")
            nc.vector.reciprocal(dinv_t[:], dsq_t[:])
            w1_t = cp.tile([DIN, HID], F32, tag="w1", name="w1")
            nc.sync.dma_start(w1_t[:], w1_d[:, :])
            w2b_t = cp.tile([HID, HID], BF16, tag="w2b", name="w2b")
            nc.sync.dma_start(w2b_t[:], w2b_d[:, :])
            w3b_t = cp.tile([HID, HID], BF16, tag="w3b", name="w3b")
            nc.sync.dma_start(w3b_t[:], w3b_d[:, :])
            wlb_t = cp.tile([HID, NCLS], BF16, tag="wlb", name="wlb")
            nc.sync.dma_start(wlb_t[:], wlb_d[:, :])
            bt = []
            for nm, d in (("b1t", b1t_d), ("b2t", b2t_d)):
                b_ = cp.tile([128, HID], F32, tag=nm, name=nm)
                nc.sync.dma_start(b_[:], d[:, :])
                bt.append(b_)
            b3t_t = cp.tile([128, HID], F32, tag="b3t", name="b3t")
            nc.sync.dma_start(b3t_t[:], b3t_d[:, :])
            blt_t = cp.tile([128, NCLS], F32, tag="blt", name="blt")
            nc.sync.dma_start(blt_t[:], blt_d[:, :])
            identb = cp.tile([128, 128], BF16, tag="identb", name="identb")
            make_identity(nc, identb[:])
            hall = cp.tile([128, BLOCKS, HID], BF16, tag="hall", name="hall")
            # tabloc[p]: this core's prescaled projected rows for layer p
            tabloc = [cp.tile([128, BLOCKS, HID], BF16, tag=f"tl{p}", name=f"tl{p}")
                      for p in range(3)]
            wnextb = [w2b_t, w3b_t]

            def bnc_write(k, b0, nb):
                row0 = b0 * 128
                tl = tabloc[k][:, b0: b0 + nb, :]
                if b0 < HB:
                    dstp = bncA[k].ap()[row0: row0 + nb * 128, :]
                else:
                    dstp = bncB[k].ap()[row0 - H1: row0 - H1 + nb * 128, :]
                nc.sync.dma_start(dstp.rearrange("(b p) h -> p b h", b=nb), tl)

            # ---- phase 1: table0 = (x @ W1) * dinv ----
            with nc.named_scope("p1"):
                for si, sw in enumerate(t.sws):
                    nb = len(sw)
                    b0 = sw[0]
                    xt = xp.tile([DIN, 4 * 128], F32, tag="xt", name="xt")
                    nc.sync.dma_start(xt[:, : nb * 128], xT_d[:, b0 * 128: (b0 + nb) * 128])
                    for jb, b in enumerate(sw):
                        ps = psw.tile([128, HID], F32, tag="psw", name="psw")
                        nc.tensor.matmul(ps[:], lhsT=xt[:, jb * 128: (jb + 1) * 128],
                                         rhs=w1_t[:], start=True, stop=True)
                        nc.scalar.activation(tabloc[0][:, b, :], ps[:], AF.Copy,
                                             scale=dinv_t[:, b: b + 1])
                    bnc_write(0, b0, nb)
                    if b0 + nb == HB:
                        ag_half(0, 0)
                # zero pad block (rows SH-128..SH live in bncB tail)
                zt = ep.tile([128, HID], BF16, tag="h", name="h")
                nc.any.memset(zt[:], 0.0)
                nc.sync.dma_start(bncB[0][H2 - 128: H2, :], zt[:])
                ag_half(0, 1)

            # ---- layers 0,1 (gather layers) ----
            state = {}
            pool_ps = [psg.tile([128, HID], F32, tag=f"psg{gb}", name=f"psg{gb}")
                       for gb in range(GB)]

            def dispatch(k, si, rviews):
                ixr = ixp.tile([128, max(4 * ix_max, 16)], I16, tag="ix", name="ix")
                c0 = icol[si][0]
                cend = icol[si][R - 1] + Lsr[si][R - 1] // 16
                nc.sync.dma_start(ixr[:, : cend - c0], gidx_d[:, c0:cend])
                dw = t.dwin[si]
                dlt = None
                if dw:
                    dlt = ixp.tile([128, dw_max], F32, tag="dlo", name="dlo")
                    d0 = t.dcol[si]
                    nc.sync.dma_start(dlt[:, :dw], dlo_d[:, d0: d0 + dw])
                gts = [None] * R
                for r in (1, 2, 3, 0):
                    L = Lsr[si][r]
                    cols = colsr[si][r]
                    gt = mps[r].tile([128, col_max, 2 * HID], BF16, tag=f"m{r}", name=f"m{r}")
                    io = icol[si][r] - c0
                    nc.gpsimd.dma_gather(
                        gt[:, :cols, :], rviews[r], ixr[:, io: io + L // 16],
                        L, L, 2 * HID, elem_step=R * HID,
                        single_packet=False, queue_num=r,
                    )
                    gts[r] = gt
                state[si] = (gts, dlt)

            def process(k, si):
                sw = t.sws[si]
                nb = len(sw)
                gts, dlt = state.pop(si)
                nfix = nb * K
                cs = []
                for r in range(R):
                    hoff = (r % 2) * HID
                    csr = wp.tile([128, 4 * HID], F32, tag=f"cs{r}", name=f"cs{r}")
                    view = gts[r][:, 0:nfix, hoff: hoff + HID].rearrange(
                        "p (b j) h -> p b h j", j=K)
                    nc.vector.tensor_reduce(
                        csr[:, : nb * HID].rearrange("p (b h) -> p b h", b=nb),
                        view, axis=AX.X, op=OP.add)
                    cs.append(csr)
                s01 = wp.tile([128, 4 * HID], F32, tag="s01", name="s01")
                nc.vector.tensor_tensor(out=s01[:, : nb * HID], in0=cs[0][:, : nb * HID],
                                        in1=cs[1][:, : nb * HID], op=OP.add)
                s23 = wp.tile([128, 4 * HID], F32, tag="s23", name="s23")
                nc.vector.tensor_tensor(out=s23[:, : nb * HID], in0=cs[2][:, : nb * HID],
                                        in1=cs[3][:, : nb * HID], op=OP.add)
                s = wp.tile([128, 4 * HID], F32, tag="s", name="s")
                nc.vector.tensor_tensor(out=s[:, : nb * HID], in0=s01[:, : nb * HID],
                                        in1=s23[:, : nb * HID], op=OP.add)

                for jb, b in enumerate(sw):
                    novf = int(ovf_chunks[b, :].sum())
                    ps = None
                    if novf:
                        ps = psb.tile([128, HID], F32, tag="psb", name="psb")
                        done = 0
                        for r in range(R):
                            ch = int(ovf_chunks[b, r])
                            if ch == 0:
                                continue
                            hoff = (r % 2) * HID
                            oc = t.ocol[(si, b, r)]
                            for j in range(ch):
                                dc = t.dcol[(si, b, r, j)] - t.dcol[si]
                                oh = ohp.tile([128, 128], BF16, tag="oh", name="oh")
                                nc.vector.tensor_scalar(
                                    oh[:], iota_t[:], dlt[:, dc: dc + 1], None,
                                    OP.is_equal)
                                nc.tensor.matmul(
                                    ps[:], lhsT=oh[:],
                                    rhs=gts[r][:, oc + j, hoff: hoff + HID],
                                    start=(done == 0), stop=(done == novf - 1))
                                done += 1
                    sl = s[:, jb * HID: (jb + 1) * HID]
                    # self-loop contribution: + tabloc[k][b]
                    a1 = ep.tile([128, HID], F32, tag="agg", name="agg")
                    nc.vector.tensor_tensor(out=a1[:], in0=sl, in1=tabloc[k][:, b, :],
                                            op=OP.add)
                    if novf:
                        a2 = ep.tile([128, HID], F32, tag="agg2", name="agg2")
                        nc.vector.tensor_tensor(out=a2[:], in0=a1[:], in1=ps[:], op=OP.add)
                        aggap = a2[:]
                    else:
                        aggap = a1[:]
                    t1 = ep.tile([128, HID], F32, tag="t1", name="t1")
                    nc.scalar.activation(t1[:], aggap, AF.Copy, scale=dinv_t[:, b: b + 1])
                    hp = ep.tile([128, HID], F32, tag="hp", name="hp")
                    nc.vector.tensor_tensor(out=hp[:], in0=t1[:], in1=bt[k][:], op=OP.add)
                    nc.scalar.activation(hall[:, b, :], hp[:], AF.Relu)

            def projwin(k, si):
                sw = t.sws[si]
                nb = len(sw)
                wgt_t = None
                if k == 1:
                    wgt_t = wgp.tile([128, 4, NG], BF16, tag="wgt", name="wgt")
                    b0 = sw[0]
                    nc.sync.dma_start(
                        wgt_t[:, :nb, :],
                        wgt_d.ap()[b0 * 128: (b0 + nb) * 128, :].rearrange(
                            "(b p) g -> p b g", b=nb))
                for jb, b in enumerate(sw):
                    pt = pst.tile([128, 128], BF16, tag="pstb", name="pstb")
                    nc.tensor.transpose(pt[:HID, :], hall[:, b, :], identb[:])
                    hT = ep.tile([HID, 128], BF16, tag="hT", name="hT")
                    nc.scalar.copy(hT[:], pt[:HID, :])
                    ps2 = psw.tile([128, HID], F32, tag="psw", name="psw")
                    nc.tensor.matmul(ps2[:], lhsT=hT[:], rhs=wnextb[k][:],
                                     start=True, stop=True)
                    nc.scalar.activation(tabloc[k + 1][:, b, :], ps2[:], AF.Copy,
                                         scale=dinv_t[:, b: b + 1])
                    if k == 1:
                        for gb in range(GB):
                            gp = min(128, NG - gb * 128)
                            nc.tensor.matmul(
                                pool_ps[gb][:gp, :],
                                lhsT=wgt_t[:, gb * 128: gb * 128 + gp],
                                rhs=tabloc[2][:, b, :],
                                start=(b == 0), stop=(b == BLOCKS - 1))
                if k == 0:
                    bnc_write(1, sw[0], nb)

            for k in range(2):
                tview = tab[k].ap().rearrange("(a b) d -> a (b d)", b=R)
                rviews = [tview[:, (r // 2) * 2 * HID: (r // 2 + 1) * 2 * HID] for r in range(R)]
                sid, _ = nc.enter_named_scope(f"L{k}", notify=False)
                D = 2
                for si in range(NW):
                    dispatch(k, si, rviews)
                    if si > 0:
                        process(k, si - 1)
                    if si - D >= 0:
                        projwin(k, si - D)
                    if k == 0 and si - D == 11:
                        ag_half(1, 0)  # blocks 0..47 projected by windows 0..11
                process(k, NW - 1)
                for si in range(NW - D, NW):
                    projwin(k, si)
                nc.leave_named_scope(f"L{k}", sid, notify=False)
                if k == 0:
                    zt = ep.tile([128, HID], BF16, tag="h", name="h")
                    nc.any.memset(zt[:], 0.0)
                    nc.sync.dma_start(bncB[1][H2 - 128: H2, :], zt[:])
                    with nc.named_scope("ag1"):
                        ag_half(1, 1)

            # ---- tail: AllReduce pooled sums, mean, +b3, linear ----
            tid, _ = nc.enter_named_scope("tail", notify=False)
            for gb in range(GB):
                gp = min(128, NG - gb * 128)
                cpt = ep.tile([128, HID], F32, tag="t1", name="t1")
                if gp < 128:
                    nc.any.memset(cpt[:], 0.0)
                nc.vector.tensor_copy(out=cpt[:gp, :], in_=pool_ps[gb][:gp, :])
                nc.sync.dma_start(pool_loc[gb * 128: (gb + 1) * 128, :], cpt[:])
            nc.gpsimd.collective_compute(
                "AllReduce", OP.add, replica_groups=groups,
                ins=[pool_loc.ap().opt()], outs=[pool_sum.ap().opt()],
            )
            for gb in range(GB):
                gp = min(128, NG - gb * 128)
                sm = ep.tile([128, HID], F32, tag="t1", name="t1")
                nc.sync.dma_start(sm[:], pool_sum[gb * 128: (gb + 1) * 128, :])
                mean = ep.tile([128, HID], F32, tag="mean", name="mean")
                nc.vector.tensor_scalar(mean[:], sm[:], invc_t[:, gb: gb + 1], None, OP.mult)
                hb = ep.tile([128, HID], BF16, tag="h", name="h")
                nc.vector.tensor_tensor(out=hb[:], in0=mean[:], in1=b3t_t[:], op=OP.add)
                pt = pst.tile([128, 128], BF16, tag="pstb", name="pstb")
                nc.tensor.transpose(pt[:HID, :], hb[:], identb[:])
                mT = ep.tile([HID, 128], BF16, tag="hT", name="hT")
                nc.scalar.copy(mT[:], pt[:HID, :])
                psf = psw.tile([128, NCLS], F32, tag="psw", name="psw")
                nc.tensor.matmul(psf[:gp, :], lhsT=mT[:, :gp], rhs=wlb_t[:], start=True, stop=True)
                of = ep.tile([128, NCLS], F32, tag="of", name="of")
                nc.vector.tensor_tensor(out=of[:gp, :], in0=psf[:gp, :], in1=blt_t[:gp, :], op=OP.add)
                nc.sync.dma_start(out_d[gb * 128: gb * 128 + gp, :], of[:gp, :])
            nc.leave_named_scope("tail", tid, notify=False)

    if os.environ.get("GCN_BUILD_ONLY", "0") != "1":
        nc.compile()
    return nc


# --------------------------------------------------------------------------
# Entry points
# --------------------------------------------------------------------------

def run_on_hw(inputs, cfg, trace=None):
    from concourse.bass_utils import run_bass_kernel_spmd

    if trace is None:
        trace = os.environ.get("GCN_TRACE", "0") == "1"
    t = preprocess(np.asarray(inputs["x"]), np.asarray(inputs["edge_index"]),
                   np.asarray(inputs["batch"]), cfg)
    in_maps = make_in_maps(
        t, *(np.asarray(inputs[k]) for k in
             ("W1", "b1", "W2", "b2", "W3", "b3", "Wlin", "blin")))
    nc = build_program(t)
    res = run_bass_kernel_spmd(nc, in_maps, core_ids=list(range(cfg["n_cores"])), trace=trace)
    run_on_hw.last = res
    return res.results[0]["out"].astype(np.float32)


def kernel(**inputs) -> np.ndarray:
    return run_on_hw(inputs, CFG_FULL)
